# revision 11
# baseline (speedup 1.0000x reference)
"""CalderaLinear Trainium2 kernel.

Computes out = x @ dequant(q).T + (x @ dequant(r).T) @ dequant(l).T + bias
with groupwise (group=128) dequantization, distributed over 8 NeuronCores
by sharding tokens (batch*seq) 8 ways and replicating the weights.

Device does: dequant (scale multiply), both matmuls, low-rank path, bias add.
Host does: sharding + lossless int->bf16 casts of the quantized values
(0..15 are exact in bf16) + fp32->bf16 cast of x (the matmul precision
choice), and the final concat of per-core output shards.
"""

import os
import sys

import numpy as np
import ml_dtypes

for _p in ("/opt/trn_rl_repo",):
    if _p not in sys.path and os.path.isdir(_p):
        sys.path.insert(0, _p)

import concourse.bass as bass
import concourse.mybir as mybir
import concourse.tile as tile
from concourse import bacc
from concourse.bass_utils import run_bass_kernel_spmd

BF16 = mybir.dt.bfloat16
F32 = mybir.dt.float32

P = 128  # partitions / group size
N_CORES = 8

# Full problem shape (hardcoded per contest contract).
B, S, D_IN, D_OUT, RANK = 4, 2048, 4096, 4096, 256
N_TOK = B * S  # 8192
T_SH = N_TOK // N_CORES  # 1024 tokens per core


def caldera_tile_kernel(tc, out, x, qv, qs, lv, ls, rv, rs, bias_, obw=512):
    """One core's program. Shapes:
    x   [T, K]  bf16   (token shard, natural layout)
    qv  [O, K]  bf16   quantized values; qs [O, K/128] f32 scales
    lv  [O, R]  bf16 ; ls [O, R/128] f32
    rv  [R, K]  bf16 ; rs [R, K/128] f32
    bias_ [1, O] f32
    out [T, O]  f32
    """
    nc = tc.nc
    T, K = x.shape
    O = qv.shape[0]
    R = lv.shape[1]
    KC = K // P   # contraction chunks (== scale groups along K)
    RC = R // P   # rank chunks (== scale groups along R)
    TS = T // P   # token subtiles (psum partition dim)
    OBW = obw     # output-feature block width (moving free dim, psum N)
    NOB = O // OBW
    OBB = OBW // P  # 128-row blocks per output block
    TH = min(512, T)  # xr moving width
    NTH = T // TH
    XSPLIT = 4    # x_T loaded in XSPLIT transposes for earlier PE start
    rings = (nc.sync, nc.sync)  # single HWDGE ring (dual-ring corrupted on HW)

    with tc.tile_pool(name="const", bufs=1) as constp, \
         tc.tile_pool(name="stage", bufs=2) as stagep, \
         tc.tile_pool(name="qtp", bufs=2) as qtp, \
         tc.tile_pool(name="outp", bufs=6) as outp, \
         tc.tile_pool(name="psp", bufs=6, space="PSUM") as psp, \
         tc.tile_pool(name="psxr", bufs=2, space="PSUM") as psxrp:

        # ---- resident tensors ----
        x_T = constp.tile([P, KC, T], BF16)    # x.T chunks: x_T[p,g,t] = x[t, g*P+p]
        r_T = constp.tile([P, KC, R], BF16)    # r.T chunks: r_T[p,g,r] = r_deq[r, g*P+p]
        l_T = constp.tile([P, RC, O], BF16)    # l.T chunks: l_T[p,c,o] = l_deq[o, c*P+p]
        xr_T = constp.tile([P, RC, T], BF16)   # xr.T chunks: xr_T[p,c,t] = xr[t, c*P+p]
        ones = constp.tile([1, P], BF16)
        nc.vector.memset(ones[:], 1.0)
        bias_bf = constp.tile([1, O], BF16)
        # SWDGE dma casts f32 -> bf16 in flight
        nc.gpsimd.dma_start(out=bias_bf[:], in_=bias_[:])

        # ---- r: load + in-place dequant + one 3D transpose per block ----
        for rb in range(RC):
            rnat = stagep.tile([P, K], BF16, tag="nat")
            rsc = stagep.tile([P, KC], F32, tag="sc")
            nc.gpsimd.dma_start(out=rnat[:], in_=rv[rb * P:(rb + 1) * P, :])
            nc.gpsimd.dma_start(out=rsc[:], in_=rs[rb * P:(rb + 1) * P, :])
            nc.vector.tensor_tensor(
                out=rnat[:].rearrange("p (g j) -> p g j", g=KC),
                in0=rnat[:].rearrange("p (g j) -> p g j", g=KC),
                in1=rsc[:, :, None].broadcast_to([P, KC, P]),
                op=mybir.AluOpType.mult,
            )
            nc.sync.dma_start(
                out=r_T[:, :, rb * P:(rb + 1) * P], in_=rnat[:],
                transpose=True,
            )

        # ---- l: load + dequant + one 3D transpose per block ----
        for lb in range(O // P):
            lnat = stagep.tile([P, R], BF16, tag="lnat")
            lsc = stagep.tile([P, RC], F32, tag="lsc")
            nc.gpsimd.dma_start(out=lnat[:], in_=lv[lb * P:(lb + 1) * P, :])
            nc.gpsimd.dma_start(out=lsc[:], in_=ls[lb * P:(lb + 1) * P, :])
            ldeq = stagep.tile([P, R], BF16, tag="ldeq")
            nc.vector.tensor_tensor(
                out=ldeq[:].rearrange("p (c j) -> p c j", c=RC),
                in0=lnat[:].rearrange("p (c j) -> p c j", c=RC),
                in1=lsc[:, :, None].broadcast_to([P, RC, P]),
                op=mybir.AluOpType.mult,
            )
            nc.sync.dma_start(
                out=l_T[:, :, lb * P:(lb + 1) * P], in_=ldeq[:],
                transpose=True,
            )

        # ---- x: transposed loads straight from DRAM (split for early start) --
        kstep = KC // XSPLIT
        for xs in range(XSPLIT):
            nc.sync.dma_start(
                out=x_T[:, xs * kstep:(xs + 1) * kstep, :],
                in_=x[:, xs * kstep * P:(xs + 1) * kstep * P],
                transpose=True,
            )

        # ---- xr.T = (x @ r_deq.T).T, computed as r_chunk @ x.T ----
        for rb in range(RC):
            for th in range(NTH):
                pxr = psxrp.tile([P, TH], F32, tag="psxr")
                for g in range(KC):
                    nc.tensor.matmul(
                        pxr[:],
                        lhsT=r_T[:, g, rb * P:(rb + 1) * P],
                        rhs=x_T[:, g, th * TH:(th + 1) * TH],
                        start=(g == 0),
                        stop=(g == KC - 1),
                    )
                nc.scalar.copy(xr_T[:, rb, th * TH:(th + 1) * TH], pxr[:])

        # ---- main: stream q (and l) blocks, accumulate in PSUM ----
        for ob in range(NOB):
            q_T = qtp.tile([P, KC, OBW], BF16, tag="qT")
            for c in range(OBB):
                b = ob * OBB + c
                ring = rings[b % 2]
                qnat = stagep.tile([P, K], BF16, tag="nat")
                qsc = stagep.tile([P, KC], F32, tag="sc")
                nc.gpsimd.dma_start(out=qnat[:], in_=qv[b * P:(b + 1) * P, :])
                nc.gpsimd.dma_start(out=qsc[:], in_=qs[b * P:(b + 1) * P, :])
                nc.vector.tensor_tensor(
                    out=qnat[:].rearrange("p (g j) -> p g j", g=KC),
                    in0=qnat[:].rearrange("p (g j) -> p g j", g=KC),
                    in1=qsc[:, :, None].broadcast_to([P, KC, P]),
                    op=mybir.AluOpType.mult,
                )
                ring.dma_start(
                    out=q_T[:, :, c * P:(c + 1) * P], in_=qnat[:],
                    transpose=True,
                )
            for t in range(TS):
                ps = psp.tile([P, OBW], F32)
                for g in range(KC):
                    nc.tensor.matmul(
                        ps[:],
                        lhsT=x_T[:, g, t * P:(t + 1) * P],
                        rhs=q_T[:, g, :],
                        start=(g == 0),
                        stop=False,
                    )
                for c in range(RC):
                    nc.tensor.matmul(
                        ps[:],
                        lhsT=xr_T[:, c, t * P:(t + 1) * P],
                        rhs=l_T[:, c, ob * OBW:(ob + 1) * OBW],
                        start=False,
                        stop=False,
                    )
                nc.tensor.matmul(
                    ps[:],
                    lhsT=ones[:],
                    rhs=bias_bf[:, ob * OBW:(ob + 1) * OBW],
                    start=False,
                    stop=True,
                )
                osb = outp.tile([P, OBW], F32)
                if t % 2 == 0:
                    nc.scalar.copy(osb[:], ps[:])
                else:
                    nc.vector.tensor_copy(osb[:], ps[:])
                rings[(ob + t) % 2].dma_start(
                    out=out[t * P:(t + 1) * P, ob * OBW:(ob + 1) * OBW],
                    in_=osb[:],
                )


def build_nc(T=T_SH, O=D_OUT, K=D_IN, R=RANK, obw=512):
    nc = bacc.Bacc("TRN2", target_bir_lowering=False, debug=False)
    x = nc.dram_tensor("x_sh", [T, K], BF16, kind="ExternalInput").ap()
    qv = nc.dram_tensor("qv", [O, K], BF16, kind="ExternalInput").ap()
    qs = nc.dram_tensor("qs", [O, K // P], F32, kind="ExternalInput").ap()
    lv = nc.dram_tensor("lv", [O, R], BF16, kind="ExternalInput").ap()
    ls = nc.dram_tensor("ls", [O, R // P], F32, kind="ExternalInput").ap()
    rv = nc.dram_tensor("rv", [R, K], BF16, kind="ExternalInput").ap()
    rs = nc.dram_tensor("rs", [R, K // P], F32, kind="ExternalInput").ap()
    bias_ = nc.dram_tensor("bias", [1, O], F32, kind="ExternalInput").ap()
    out = nc.dram_tensor("out", [T, O], F32, kind="ExternalOutput").ap()
    with tile.TileContext(nc) as tc:
        caldera_tile_kernel(tc, out, x, qv, qs, lv, ls, rv, rs, bias_,
                            obw=obw)
    nc.compile()
    return nc


def make_in_maps(x, q_values, q_scales, l_values, l_scales, r_values, r_scales,
                 bias):
    bf16 = ml_dtypes.bfloat16
    xf = np.ascontiguousarray(
        np.asarray(x, dtype=np.float32).reshape(N_TOK, D_IN)
    ).astype(bf16)
    qv = np.asarray(q_values).astype(bf16)  # ints 0..15: exact
    lv = np.asarray(l_values).astype(bf16)
    rv = np.asarray(r_values).astype(bf16)
    qs = np.ascontiguousarray(np.asarray(q_scales, dtype=np.float32))
    ls = np.ascontiguousarray(np.asarray(l_scales, dtype=np.float32))
    rs = np.ascontiguousarray(np.asarray(r_scales, dtype=np.float32))
    b = np.ascontiguousarray(
        np.asarray(bias, dtype=np.float32).reshape(1, D_OUT)
    )
    in_maps = []
    for i in range(N_CORES):
        in_maps.append({
            "x_sh": np.ascontiguousarray(xf[i * T_SH:(i + 1) * T_SH]),
            "qv": qv, "qs": qs, "lv": lv, "ls": ls,
            "rv": rv, "rs": rs, "bias": b,
        })
    return in_maps


_NC_CACHE = {}


def _get_nc():
    if "nc" not in _NC_CACHE:
        _NC_CACHE["nc"] = build_nc()
    return _NC_CACHE["nc"]


def run(inputs, trace=False, tmpdir=None):
    nc = _get_nc()
    in_maps = make_in_maps(**inputs)
    res = run_bass_kernel_spmd(
        nc, in_maps, list(range(N_CORES)), trace=trace, tmpdir=tmpdir
    )
    shards = [np.asarray(res.results[i]["out"]) for i in range(N_CORES)]
    full = np.concatenate(shards, axis=0).reshape(B, S, D_OUT)
    return full.astype(np.float32), res


def kernel(**inputs) -> np.ndarray:
    out, _ = run(inputs, trace=False)
    return out


# revision 12
# speedup vs baseline: 1.1055x; 1.1055x over previous
"""CalderaLinear Trainium2 kernel.

Computes out = x @ dequant(q).T + (x @ dequant(r).T) @ dequant(l).T + bias
with groupwise (group=128) dequantization, distributed over 8 NeuronCores
by sharding tokens (batch*seq) 8 ways and replicating the weights.

Device does: dequant (scale multiply), both matmuls, low-rank path, bias add.
Host does: sharding + lossless int->bf16 casts of the quantized values
(0..15 are exact in bf16) + fp32->bf16 cast of x (the matmul precision
choice), and the final concat of per-core output shards.
"""

import os
import sys

import numpy as np
import ml_dtypes

for _p in ("/opt/trn_rl_repo",):
    if _p not in sys.path and os.path.isdir(_p):
        sys.path.insert(0, _p)

import concourse.bass as bass
import concourse.mybir as mybir
import concourse.tile as tile
from concourse import bacc
from concourse.bass_utils import run_bass_kernel_spmd

BF16 = mybir.dt.bfloat16
F32 = mybir.dt.float32

P = 128  # partitions / group size
N_CORES = 8

# Full problem shape (hardcoded per contest contract).
B, S, D_IN, D_OUT, RANK = 4, 2048, 4096, 4096, 256
N_TOK = B * S  # 8192
T_SH = N_TOK // N_CORES  # 1024 tokens per core


def caldera_tile_kernel(tc, out, x, qv, qs, lv, ls, rv, rs, bias_, obw=512):
    """One core's program. Shapes:
    x   [T, K]  bf16   (token shard, natural layout)
    qv  [O, K]  bf16   quantized values; qs [O, K/128] f32 scales
    lv  [O, R]  bf16 ; ls [O, R/128] f32
    rv  [R, K]  bf16 ; rs [R, K/128] f32
    bias_ [1, O] f32
    out [T, O]  f32
    """
    nc = tc.nc
    T, K = x.shape
    O = qv.shape[0]
    R = lv.shape[1]
    KC = K // P   # contraction chunks (== scale groups along K)
    RC = R // P   # rank chunks (== scale groups along R)
    TS = T // P   # token subtiles (psum partition dim)
    OBW = obw     # output-feature block width (moving free dim, psum N)
    NOB = O // OBW
    OBB = OBW // P  # 128-row blocks per output block
    TH = min(512, T)  # xr moving width
    NTH = T // TH
    XSPLIT = 4    # x_T loaded in XSPLIT transposes for earlier PE start
    rings = (nc.sync, nc.sync)  # single HWDGE ring (dual-ring corrupted on HW)

    with tc.tile_pool(name="const", bufs=1) as constp, \
         tc.tile_pool(name="stage", bufs=2) as stagep, \
         tc.tile_pool(name="qtp", bufs=2) as qtp, \
         tc.tile_pool(name="outp", bufs=6) as outp, \
         tc.tile_pool(name="psp", bufs=6, space="PSUM") as psp, \
         tc.tile_pool(name="psxr", bufs=2, space="PSUM") as psxrp:

        # ---- resident tensors ----
        x_T = constp.tile([P, KC, T], BF16)    # x.T chunks: x_T[p,g,t] = x[t, g*P+p]
        r_T = constp.tile([P, KC, R], BF16)    # r.T chunks: r_T[p,g,r] = r_deq[r, g*P+p]
        l_T = constp.tile([P, RC, O], BF16)    # l.T chunks: l_T[p,c,o] = l_deq[o, c*P+p]
        xr_T = constp.tile([P, RC, T], BF16)   # xr.T chunks: xr_T[p,c,t] = xr[t, c*P+p]
        ones = constp.tile([1, P], BF16)
        nc.vector.memset(ones[:], 1.0)
        bias_bf = constp.tile([1, O], BF16)
        # SWDGE dma casts f32 -> bf16 in flight
        nc.gpsimd.dma_start(out=bias_bf[:], in_=bias_[:])

        # ---- r: load + in-place dequant + one 3D transpose per block ----
        for rb in range(RC):
            rnat = stagep.tile([P, K], BF16, tag="nat")
            rsc = stagep.tile([P, KC], F32, tag="sc")
            nc.sync.dma_start(out=rnat[:], in_=rv[rb * P:(rb + 1) * P, :])
            nc.sync.dma_start(out=rsc[:], in_=rs[rb * P:(rb + 1) * P, :])
            nc.vector.tensor_tensor(
                out=rnat[:].rearrange("p (g j) -> p g j", g=KC),
                in0=rnat[:].rearrange("p (g j) -> p g j", g=KC),
                in1=rsc[:, :, None].broadcast_to([P, KC, P]),
                op=mybir.AluOpType.mult,
            )
            nc.sync.dma_start(
                out=r_T[:, :, rb * P:(rb + 1) * P], in_=rnat[:],
                transpose=True,
            )

        # ---- l: load + dequant + one 3D transpose per block ----
        for lb in range(O // P):
            lnat = stagep.tile([P, R], BF16, tag="lnat")
            lsc = stagep.tile([P, RC], F32, tag="lsc")
            nc.sync.dma_start(out=lnat[:], in_=lv[lb * P:(lb + 1) * P, :])
            nc.sync.dma_start(out=lsc[:], in_=ls[lb * P:(lb + 1) * P, :])
            ldeq = stagep.tile([P, R], BF16, tag="ldeq")
            nc.vector.tensor_tensor(
                out=ldeq[:].rearrange("p (c j) -> p c j", c=RC),
                in0=lnat[:].rearrange("p (c j) -> p c j", c=RC),
                in1=lsc[:, :, None].broadcast_to([P, RC, P]),
                op=mybir.AluOpType.mult,
            )
            nc.sync.dma_start(
                out=l_T[:, :, lb * P:(lb + 1) * P], in_=ldeq[:],
                transpose=True,
            )

        # ---- x: transposed loads straight from DRAM (split for early start) --
        kstep = KC // XSPLIT
        for xs in range(XSPLIT):
            nc.sync.dma_start(
                out=x_T[:, xs * kstep:(xs + 1) * kstep, :],
                in_=x[:, xs * kstep * P:(xs + 1) * kstep * P],
                transpose=True,
            )

        # ---- xr.T = (x @ r_deq.T).T, computed as r_chunk @ x.T ----
        for rb in range(RC):
            for th in range(NTH):
                pxr = psxrp.tile([P, TH], F32, tag="psxr")
                for g in range(KC):
                    nc.tensor.matmul(
                        pxr[:],
                        lhsT=r_T[:, g, rb * P:(rb + 1) * P],
                        rhs=x_T[:, g, th * TH:(th + 1) * TH],
                        start=(g == 0),
                        stop=(g == KC - 1),
                    )
                nc.scalar.copy(xr_T[:, rb, th * TH:(th + 1) * TH], pxr[:])

        # ---- main: stream q (and l) blocks, accumulate in PSUM ----
        for ob in range(NOB):
            q_T = qtp.tile([P, KC, OBW], BF16, tag="qT")
            for c in range(OBB):
                b = ob * OBB + c
                ring = rings[b % 2]
                qnat = stagep.tile([P, K], BF16, tag="nat")
                qsc = stagep.tile([P, KC], F32, tag="sc")
                ring.dma_start(out=qnat[:], in_=qv[b * P:(b + 1) * P, :])
                ring.dma_start(out=qsc[:], in_=qs[b * P:(b + 1) * P, :])
                nc.vector.tensor_tensor(
                    out=qnat[:].rearrange("p (g j) -> p g j", g=KC),
                    in0=qnat[:].rearrange("p (g j) -> p g j", g=KC),
                    in1=qsc[:, :, None].broadcast_to([P, KC, P]),
                    op=mybir.AluOpType.mult,
                )
                ring.dma_start(
                    out=q_T[:, :, c * P:(c + 1) * P], in_=qnat[:],
                    transpose=True,
                )
            for t in range(TS):
                ps = psp.tile([P, OBW], F32)
                for g in range(KC):
                    nc.tensor.matmul(
                        ps[:],
                        lhsT=x_T[:, g, t * P:(t + 1) * P],
                        rhs=q_T[:, g, :],
                        start=(g == 0),
                        stop=False,
                    )
                for c in range(RC):
                    nc.tensor.matmul(
                        ps[:],
                        lhsT=xr_T[:, c, t * P:(t + 1) * P],
                        rhs=l_T[:, c, ob * OBW:(ob + 1) * OBW],
                        start=False,
                        stop=False,
                    )
                nc.tensor.matmul(
                    ps[:],
                    lhsT=ones[:],
                    rhs=bias_bf[:, ob * OBW:(ob + 1) * OBW],
                    start=False,
                    stop=True,
                )
                osb = outp.tile([P, OBW], F32)
                nc.scalar.copy(osb[:], ps[:])
                rings[(ob + t) % 2].dma_start(
                    out=out[t * P:(t + 1) * P, ob * OBW:(ob + 1) * OBW],
                    in_=osb[:],
                )


def build_nc(T=T_SH, O=D_OUT, K=D_IN, R=RANK, obw=512):
    nc = bacc.Bacc("TRN2", target_bir_lowering=False, debug=False)
    x = nc.dram_tensor("x_sh", [T, K], BF16, kind="ExternalInput").ap()
    qv = nc.dram_tensor("qv", [O, K], BF16, kind="ExternalInput").ap()
    qs = nc.dram_tensor("qs", [O, K // P], F32, kind="ExternalInput").ap()
    lv = nc.dram_tensor("lv", [O, R], BF16, kind="ExternalInput").ap()
    ls = nc.dram_tensor("ls", [O, R // P], F32, kind="ExternalInput").ap()
    rv = nc.dram_tensor("rv", [R, K], BF16, kind="ExternalInput").ap()
    rs = nc.dram_tensor("rs", [R, K // P], F32, kind="ExternalInput").ap()
    bias_ = nc.dram_tensor("bias", [1, O], F32, kind="ExternalInput").ap()
    out = nc.dram_tensor("out", [T, O], F32, kind="ExternalOutput").ap()
    with tile.TileContext(nc) as tc:
        caldera_tile_kernel(tc, out, x, qv, qs, lv, ls, rv, rs, bias_,
                            obw=obw)
    nc.compile()
    return nc


def make_in_maps(x, q_values, q_scales, l_values, l_scales, r_values, r_scales,
                 bias):
    bf16 = ml_dtypes.bfloat16
    xf = np.ascontiguousarray(
        np.asarray(x, dtype=np.float32).reshape(N_TOK, D_IN)
    ).astype(bf16)
    qv = np.asarray(q_values).astype(bf16)  # ints 0..15: exact
    lv = np.asarray(l_values).astype(bf16)
    rv = np.asarray(r_values).astype(bf16)
    qs = np.ascontiguousarray(np.asarray(q_scales, dtype=np.float32))
    ls = np.ascontiguousarray(np.asarray(l_scales, dtype=np.float32))
    rs = np.ascontiguousarray(np.asarray(r_scales, dtype=np.float32))
    b = np.ascontiguousarray(
        np.asarray(bias, dtype=np.float32).reshape(1, D_OUT)
    )
    in_maps = []
    for i in range(N_CORES):
        in_maps.append({
            "x_sh": np.ascontiguousarray(xf[i * T_SH:(i + 1) * T_SH]),
            "qv": qv, "qs": qs, "lv": lv, "ls": ls,
            "rv": rv, "rs": rs, "bias": b,
        })
    return in_maps


_NC_CACHE = {}


def _get_nc():
    if "nc" not in _NC_CACHE:
        _NC_CACHE["nc"] = build_nc()
    return _NC_CACHE["nc"]


def run(inputs, trace=False, tmpdir=None):
    nc = _get_nc()
    in_maps = make_in_maps(**inputs)
    res = run_bass_kernel_spmd(
        nc, in_maps, list(range(N_CORES)), trace=trace, tmpdir=tmpdir
    )
    shards = [np.asarray(res.results[i]["out"]) for i in range(N_CORES)]
    full = np.concatenate(shards, axis=0).reshape(B, S, D_OUT)
    return full.astype(np.float32), res


def kernel(**inputs) -> np.ndarray:
    out, _ = run(inputs, trace=False)
    return out
